# revision 24
# baseline (speedup 1.0000x reference)
"""Distributed Trainium2 Bass kernel for a single attention head.

Problem (hardcoded): q,k,v [4, 4096, 1024] f32, Wq/Wk/Wv [1024, 64] f32,
attn_mask [4096, 4096] bool (True = keep).  out[b] = softmax(mask(q Wq (k Wk)^T) / 8) (v Wv).

Sharding: 8 cores; core c -> batch c//2, parity par = c%2.  The k/v rows of
the batch are split by 128-row k-tile parity: core par owns global k-tiles
{2i+par}.  Each core computes, for every 512-row query chunk j, the partial
(unnormalized) attention output sum_k exp(s)*v and the partial denominator
over ITS k-tiles only.  The host sums the two cores' partials and normalizes
(flash-attention style additive combine; no on-device collectives needed).
This balances the causal work exactly (72 score tiles per core) and avoids
duplicate k/v loads; all device data is staged host-side as bf16.

On-device layout / scheduling tricks:
- All inputs are host-pre-tiled into the exact [128-partition, d-tile, col]
  SBUF layouts so every DMA is fully contiguous (8KB runs per partition),
  and every staged x chunk gets its own SBUF buffer so all input DMAs issue
  up front and the HBM stream never stalls on compute progress.
- Wq / Wk are duplicated column-wise in the packed weight so the projections
  produce qh / kh replicated in both partition halves.  Score matmuls have
  K=64; even/odd local k-tiles are stored in partition halves 0-63 / 64-127,
  so each beat's two score matmuls land in disjoint PE row-groups
  (tile_position (0,0) / (64,0)) and run CONCURRENTLY in the array.
- exp() runs on 1024-wide spans (two score tiles) to amortize ACT overhead;
  within a DMA window the beats of two query chunks alternate so their
  S -> exp -> PV pipelines hide the activation latency.
- q chunks are DMAed big-chunks-first (q2,q3 | q4,q5 | q6,q7,q1 | q0), so
  attention work unlocks uniformly against the DMA stream and only ~4 beats
  remain after the last byte lands; projection matmuls are interleaved
  between beats (offset past each round's start so not-yet-arrived data
  never stalls the in-order PE queue).
- Output partials are written as bf16 and drained via the idle gpsimd DMA
  queue (a sync-queue drain would block later input DMA issue).
- The boolean mask is block-analyzed on the host: fully-kept 512x128 blocks
  need no mask work, fully-dropped blocks are skipped at compile time,
  partially-kept blocks multiply the exp() output by a 0/1 tile from a tiny
  deduplicated table (2 distinct tiles for a causal mask).
"""

import os
import sys

sys.path.insert(0, "/opt/trn_rl_repo")

import numpy as np
import ml_dtypes

import concourse.bass as bass
import concourse.mybir as mybir
import concourse.tile as tile
from concourse import bacc
from concourse.bass_utils import run_bass_kernel_spmd
from concourse.masks import make_identity

F32 = mybir.dt.float32
BF16 = mybir.dt.bfloat16
BF16_NP = ml_dtypes.bfloat16

N_CORES = 8
B, T, D, H = 4, 4096, 1024, 64
P = 128                      # partitions / k-tile rows
QC = 512                     # query chunk width
NJ = T // QC                 # 8 query chunks
GT = T // P                  # 32 global k-tiles
LT = GT // 2                 # 16 local (per-parity) k-tiles
D_TILES = D // P             # 8
KVW = 512                    # k/v projection chunk width (4 local tiles)
NKV = LT * P // KVW          # 4 kv chunks / emission blocks
# DMA window contents: kv chunks front-loaded, q streamed one chunk per
# window so attention work unlocks uniformly against the DMA stream.
WIN_DMA = [
    [("w",), ("k", 0), ("v", 0), ("q", 2), ("q", 3)],
    [("masks",), ("k", 1), ("v", 1), ("q", 4), ("q", 5)],
    [("k", 2), ("v", 2), ("q", 6), ("q", 7), ("q", 1)],
    [("k", 3), ("v", 3), ("q", 0)],
    [],
]
NWIN = len(WIN_DMA)
KVWIN = [0, 1, 2, 3]             # window whose thunks project kv chunk c
QWIN = [3, 2, 0, 0, 1, 1, 2, 2]  # window whose thunks project q chunk j

LAST_RESULT = None           # test harness reads exec_time_ns from here
_CACHE = {}


def _schedule(mask):
    """Per query chunk j: the list of local k-tile indices both parity cores
    process (compile-time), and per entry the mask-table slot to multiply
    with (None = block fully kept for both parities)."""
    m = mask.reshape(NJ, QC, GT, P)
    blk_any = m.any(axis=(1, 3))   # [j, g]
    blk_all = m.all(axis=(1, 3))
    tidx, mslot, slots = [], [], {}
    for j in range(NJ):
        idxs, ms = [], []
        for i in range(LT):
            g0, g1 = 2 * i, 2 * i + 1
            if not (blk_any[j, g0] or blk_any[j, g1]):
                continue
            idxs.append(i)
            if blk_all[j, g0] and blk_all[j, g1]:
                ms.append(None)
            else:
                key = (mask[j * QC:(j + 1) * QC, g0 * P:(g0 + 1) * P].tobytes(),
                       mask[j * QC:(j + 1) * QC, g1 * P:(g1 + 1) * P].tobytes())
                ms.append(slots.setdefault(key, len(slots)))
        tidx.append(tuple(idxs))
        mslot.append(tuple(ms))
    return tuple(tidx), tuple(mslot), slots


def _mask_tables(mask, tidx, mslot, n_slots):
    """[2][n_slots, 128, 512] bf16 0/1 tiles (per parity)."""
    mp = [np.zeros((max(1, n_slots), P, QC), BF16_NP) for _ in range(2)]
    done = set()
    for j in range(NJ):
        for pos, i in enumerate(tidx[j]):
            s = mslot[j][pos]
            if s is None or s in done:
                continue
            done.add(s)
            for par in range(2):
                g = 2 * i + par
                blk = mask[j * QC:(j + 1) * QC, g * P:(g + 1) * P]
                mp[par][s] = blk.T.astype(BF16_NP)
    return mp


def _beat_blocks(tidx):
    """Assign attention beats (j, ii) to emission windows by data readiness;
    drains follow each chunk's last beat.  Falls back to chunk-sequential
    emission if the readiness-ordered schedule would need >3 concurrent
    PSUM accumulators."""
    ext = [len(t) for t in tidx]
    nbeats = [(e + 1) // 2 for e in ext]

    def win_of(j, ii):
        tiles = tidx[j][ii:ii + 2]
        return max(KVWIN[max(tiles) // (KVW // P)], QWIN[j])

    def entries_sorted():
        beats = []
        for j in range(NJ):
            for ii in range(0, ext[j], 2):
                beats.append((win_of(j, ii), j, ii))
        # within a window: finish already-open chunks first (frees their
        # PSUM accumulator before new chunks open), then alternate new
        # chunks ii-major so their S/exp/PV pipelines interleave
        fw = {}
        for w, j, ii in beats:
            fw[j] = min(fw.get(j, w), w)
        beats.sort(key=lambda t: (t[0], 0 if fw[t[1]] < t[0] else 1,
                                  t[2], t[1]))
        blocks = [[] for _ in range(NWIN)]
        seen = {j: 0 for j in range(NJ)}
        for w, j, ii in beats:
            blocks[w].append(("beat", j, ii))
            seen[j] += 1
            if seen[j] == nbeats[j]:
                blocks[w].append(("drain", j))
        for j in range(NJ):
            if ext[j] == 0:
                blocks[0].append(("zero", j))
                blocks[0].append(("drain", j))
        return blocks

    def ring_ok(blocks, ring=3):
        order = [e for b in blocks for e in b]
        open_order, drains = [], []
        for e in order:
            if e[0] in ("beat", "zero") and e[1] not in open_order:
                open_order.append(e[1])
                if len(open_order) > ring:
                    victim = open_order[len(open_order) - 1 - ring]
                    if victim not in drains:
                        return False
            elif e[0] == "drain":
                drains.append(e[1])
        return True

    blocks = entries_sorted()
    if ring_ok(blocks):
        return blocks
    # fallback: all beats of a chunk in the window where its last tile lands
    blocks = [[] for _ in range(NWIN)]
    for j in range(NJ):
        if ext[j] == 0:
            blocks[0] += [("zero", j), ("drain", j)]
            continue
        w = max(KVWIN[max(tidx[j]) // (KVW // P)], QWIN[j])
        for ii in range(0, ext[j], 2):
            blocks[w].append(("beat", j, ii))
        blocks[w].append(("drain", j))
    return blocks


def _build(tidx, mslot, n_slots):
    n_mask = max(1, n_slots)
    nc = bacc.Bacc("TRN2", target_bir_lowering=False, debug=False,
                   num_devices=N_CORES)
    qT = nc.dram_tensor("qT", [NJ, P, D_TILES, QC], BF16,
                        kind="ExternalInput")
    kT = nc.dram_tensor("kT", [NKV, P, D_TILES, KVW], BF16,
                        kind="ExternalInput")
    vT = nc.dram_tensor("vT", [NKV, P, D_TILES, KVW], BF16,
                        kind="ExternalInput")
    w = nc.dram_tensor("w", [P, D_TILES, 5 * H], BF16, kind="ExternalInput")
    maskp = nc.dram_tensor("maskp", [P, n_mask, QC], BF16,
                           kind="ExternalInput")
    out = nc.dram_tensor("out", [NJ, H + 1, QC], BF16,
                     kind="ExternalOutput")

    Exp = mybir.ActivationFunctionType.Exp
    blocks = _beat_blocks(tidx)

    with tile.TileContext(nc) as tc:
        with (
            tc.tile_pool(name="const", bufs=1) as cpool,
            tc.tile_pool(name="proj", bufs=1) as projpool,
        ):
            w_sb = cpool.tile([P, D_TILES, 5 * H], BF16)
            msk = cpool.tile([P, n_mask, QC], BF16)
            ident = cpool.tile([P, P], F32)

            qhT = projpool.tile([P, T], BF16, tag="qhT")      # qh in both halves
            khT = projpool.tile([P, LT // 2, P], BF16, tag="khT")
            vh = projpool.tile([P, LT, H + 1], BF16, tag="vh")

            with (
                tc.tile_pool(name="xs", bufs=16) as xpool,
                tc.tile_pool(name="pp", bufs=1, space="PSUM") as pppool,
                tc.tile_pool(name="sp", bufs=2, space="PSUM") as spool,
                tc.tile_pool(name="oac", bufs=3, space="PSUM") as opool,
                tc.tile_pool(name="vt", bufs=2) as vtpool,
                tc.tile_pool(name="pt", bufs=4) as ppool,
                tc.tile_pool(name="ost", bufs=2) as ostpool,
            ):
                oaccs = {}

                # ---------- emitter thunks ----------
                def dma_x(src, idx):
                    def go():
                        xt = xpool.tile([P, D_TILES, QC], BF16, tag="x",
                                        name="xt")
                        nc.sync.dma_start(out=xt[:], in_=src.ap()[idx])
                        return xt
                    return go

                def proj_thunks(xt_ref, wlo, whi, m_parts, out_cb, width):
                    """8 matmul thunks accumulating [m_parts, width] then a
                    finisher callback on the psum tile."""
                    state = {}
                    def mk(dt_):
                        def go():
                            if dt_ == 0:
                                state["ps"] = pppool.tile(
                                    [m_parts, width], F32, tag="pp", name="ps")
                            nc.tensor.matmul(
                                state["ps"][:], lhsT=w_sb[:, dt_, wlo:whi],
                                rhs=state["xt"][:, dt_, :],
                                start=(dt_ == 0), stop=(dt_ == D_TILES - 1))
                        return go
                    def first():
                        state["xt"] = xt_ref()
                    thunks = []
                    for dt_ in range(D_TILES):
                        if dt_ == 0:
                            g = mk(0)
                            thunks.append(lambda g=g: (first(), g()))
                        else:
                            thunks.append(mk(dt_))
                    thunks.append(lambda: out_cb(state["ps"]))
                    return thunks

                def q_finish(j):
                    def go(ps):
                        nc.vector.tensor_copy(
                            out=qhT[:, j * QC:(j + 1) * QC], in_=ps[:])
                    return go

                def k_finish(c):
                    def go(ps):
                        for t in range(2):
                            sl = 2 * c + t
                            nc.vector.tensor_copy(
                                out=khT[0:H, sl, :],
                                in_=ps[0:H, 2 * t * P:(2 * t + 1) * P])
                            nc.vector.tensor_copy(
                                out=khT[H:P, sl, :],
                                in_=ps[H:P, (2 * t + 1) * P:(2 * t + 2) * P])
                    return go

                def v_finish(c):
                    def go(ps):
                        vtmp = vtpool.tile([H + 1, KVW], F32, tag="vt",
                                           name="vtmp")
                        nc.vector.tensor_copy(out=vtmp[0:H, :], in_=ps[:])
                        nc.vector.memset(vtmp[H:H + 1, :], 1.0)
                        for tt in range(KVW // P):
                            tp = pppool.tile([P, H + 1], F32, tag="pp",
                                             name="tp")
                            nc.tensor.transpose(
                                tp[:], vtmp[:, tt * P:(tt + 1) * P],
                                ident[0:H + 1, 0:H + 1])
                            nc.vector.tensor_copy(
                                out=vh[:, c * (KVW // P) + tt, :], in_=tp[:])
                    return go

                def emit_beat(j, ii):
                    tiles = tidx[j][ii:ii + 2]
                    pw = len(tiles)
                    ext = len(tidx[j])
                    sp = spool.tile([P, 2 * QC], F32, tag="S", name="sp")
                    for u, i in enumerate(tiles):
                        half = (i % 2) * H
                        nc.tensor.matmul(
                            sp[:, u * QC:(u + 1) * QC],
                            lhsT=khT[half:half + H, i // 2, :],
                            rhs=qhT[half:half + H, j * QC:(j + 1) * QC],
                            start=True, stop=True)
                    pt = ppool.tile([P, 2 * QC], BF16, tag="p", name="pt")
                    nc.scalar.activation(
                        out=pt[:, 0:pw * QC], in_=sp[:, 0:pw * QC],
                        func=Exp, scale=0.125)
                    for u in range(pw):
                        s = mslot[j][ii + u]
                        if s is not None:
                            nc.vector.tensor_mul(
                                pt[:, u * QC:(u + 1) * QC],
                                pt[:, u * QC:(u + 1) * QC],
                                msk[:, s, :])
                    def pv():
                        if ii == 0:
                            oaccs[j] = opool.tile([H + 1, QC], F32,
                                                  tag="oacc", name="oacc")
                        for u, i in enumerate(tiles):
                            nc.tensor.matmul(
                                oaccs[j][:],
                                lhsT=vh[:, i, :],
                                rhs=pt[:, u * QC:(u + 1) * QC],
                                start=(ii + u == 0),
                                stop=(ii + u == ext - 1))
                    return pv

                def emit_drain(j):
                    # drain via the idle gpsimd queue: a sync-queue dma_start
                    # would wait on the result sem and stall later input DMAs
                    # behind it (the sync sequencer is in-order)
                    ost = ostpool.tile([H + 1, QC], BF16, tag="ost",
                                       name="ost")
                    nc.vector.tensor_copy(out=ost[:], in_=oaccs[j][:])
                    nc.gpsimd.dma_start(out=out.ap()[j], in_=ost[:])

                # ---------- emission ----------
                make_identity(nc, ident[:])

                prev_beats = []       # beats of window r-1, emitted in round r
                for r in range(NWIN + 1):
                    # DMAs + projection thunks for this window's data
                    thunks = []
                    for ent in (WIN_DMA[r] if r < NWIN else []):
                        if ent[0] == "w":
                            nc.sync.dma_start(out=w_sb[:], in_=w.ap())
                        elif ent[0] == "masks":
                            nc.sync.dma_start(out=msk[:], in_=maskp.ap())
                        elif ent[0] == "k":
                            c = ent[1]
                            kx = dma_x(kT, c)()
                            thunks += proj_thunks(
                                lambda kx=kx: kx, 2 * H, 4 * H, P,
                                k_finish(c), KVW)
                        elif ent[0] == "v":
                            c = ent[1]
                            vx = dma_x(vT, c)()
                            thunks += proj_thunks(
                                lambda vx=vx: vx, 4 * H, 5 * H, H,
                                v_finish(c), KVW)
                        else:
                            j = ent[1]
                            qx = dma_x(qT, j)()
                            thunks += proj_thunks(
                                lambda qx=qx: qx, 0, 2 * H, P,
                                q_finish(j), QC)
                    # interleave previous window's beats with this one's proj
                    nb = max(1, len([e for e in prev_beats if e[0] == "beat"]))
                    skip = nb // 2   # this round's thunks wait on DMA still
                    ti = 0           # in flight; don't let them stall beats
                    bi = 0
                    for e in prev_beats:
                        if e[0] == "beat":
                            pv = emit_beat(e[1], e[2])
                            bi += 1
                            hi = (len(thunks) * max(0, bi - skip)
                                  // max(1, nb - skip))
                            while ti < hi:
                                thunks[ti]()
                                ti += 1
                            pv()
                        elif e[0] == "zero":
                            oaccs[e[1]] = opool.tile([H + 1, QC], F32,
                                                     tag="oacc", name="oacc")
                            nc.vector.memset(oaccs[e[1]][:], 0.0)
                        else:
                            emit_drain(e[1])
                    while ti < len(thunks):
                        thunks[ti]()
                        ti += 1
                    prev_beats = blocks[r] if r < NWIN else []

    nc.compile()
    return nc


def _get_nc(key, tidx, mslot, n_slots):
    if key not in _CACHE:
        _CACHE[key] = _build(tidx, mslot, n_slots)
    return _CACHE[key]


def _tile_x(x2d, nchunks, width):
    """[D, nchunks*width] -> [nchunks, P, D_TILES, width] contiguous."""
    return np.ascontiguousarray(
        x2d.reshape(D_TILES, P, nchunks, width).transpose(2, 1, 0, 3))


def _make_in_maps(q, k, v, wcat, mp):
    cols = [np.concatenate(
        [np.arange((2 * i + par) * P, (2 * i + par + 1) * P)
         for i in range(LT)]) for par in range(2)]
    in_maps = []
    for c_ in range(N_CORES):
        b, par = divmod(c_, 2)
        qTb = _tile_x(q[b].T.astype(BF16_NP), NJ, QC)
        kTb = _tile_x(k[b].T[:, cols[par]].astype(BF16_NP), NKV, KVW)
        vTb = _tile_x(v[b].T[:, cols[par]].astype(BF16_NP), NKV, KVW)
        in_maps.append({
            "qT": qTb, "kT": kTb, "vT": vTb, "w": wcat,
            "maskp": np.ascontiguousarray(mp[par].transpose(1, 0, 2)),
        })
    return in_maps


def _gather_out(results):
    outp = np.empty((B, T, H), np.float32)
    for b in range(B):
        acc = (results[2 * b]["out"].astype(np.float32)
               + results[2 * b + 1]["out"].astype(np.float32))
        num = acc[:, 0:H, :]
        den = acc[:, H, :]
        outp[b] = (np.moveaxis(num, 1, 2) / den[:, :, None]).reshape(T, H)
    return outp


def kernel(q, k, v, Wq, Wk, Wv, attn_mask):
    global LAST_RESULT
    q = np.asarray(q, dtype=np.float32)
    k = np.asarray(k, dtype=np.float32)
    v = np.asarray(v, dtype=np.float32)
    mask = np.asarray(attn_mask).astype(bool)
    Wq = np.asarray(Wq, np.float32)
    Wk = np.asarray(Wk, np.float32)
    Wv = np.asarray(Wv, np.float32)
    # [Wq|Wq|Wk|Wk|Wv]: duplicated halves put qh/kh in both partition halves
    wcat = np.concatenate([Wq, Wq, Wk, Wk, Wv], axis=1).astype(BF16_NP)
    wcat = np.ascontiguousarray(
        wcat.reshape(D_TILES, P, 5 * H).transpose(1, 0, 2))

    tidx, mslot, slots = _schedule(mask)
    key = (tidx, mslot, len(slots))
    nc = _get_nc(key, tidx, mslot, len(slots))
    mp = _mask_tables(mask, tidx, mslot, len(slots))
    in_maps = _make_in_maps(q, k, v, wcat, mp)

    res = run_bass_kernel_spmd(
        nc, in_maps, core_ids=list(range(N_CORES)),
        trace=bool(os.environ.get("KBENCH_TRACE")))
    LAST_RESULT = res
    return _gather_out(res.results)


# revision 25
# speedup vs baseline: 1.1954x; 1.1954x over previous
"""Distributed Trainium2 Bass kernel for a single attention head.

Problem (hardcoded): q,k,v [4, 4096, 1024] f32, Wq/Wk/Wv [1024, 64] f32,
attn_mask [4096, 4096] bool (True = keep).  out[b] = softmax(mask(q Wq (k Wk)^T) / 8) (v Wv).

Sharding: 8 cores; core c -> batch c//2, parity par = c%2.  The k/v rows of
the batch are split by 128-row k-tile parity: core par owns global k-tiles
{2i+par}.  Each core computes, for every 512-row query chunk j, the partial
(unnormalized) attention output sum_k exp(s)*v and the partial denominator
over ITS k-tiles only.  The host sums the two cores' partials and normalizes
(flash-attention style additive combine; no on-device collectives needed).
This balances the causal work exactly (72 score tiles per core) and avoids
duplicate k/v loads; all device data is staged host-side as bf16.

On-device layout / scheduling tricks:
- All inputs are host-pre-tiled into the exact [128-partition, d-tile, col]
  SBUF layouts so every DMA is fully contiguous (8KB runs per partition),
  and every staged x chunk gets its own SBUF buffer so all input DMAs issue
  up front and the HBM stream never stalls on compute progress.
- Wq / Wk are duplicated column-wise in the packed weight so the projections
  produce qh / kh replicated in both partition halves.  Score matmuls have
  K=64; even/odd local k-tiles are stored in partition halves 0-63 / 64-127,
  so each beat's two score matmuls land in disjoint PE row-groups
  (tile_position (0,0) / (64,0)) and run CONCURRENTLY in the array.
- exp() runs on 1024-wide spans (two score tiles) to amortize ACT overhead;
  within a DMA window the beats of two query chunks alternate so their
  S -> exp -> PV pipelines hide the activation latency.
- q chunks are DMAed big-chunks-first (q2,q3 | q4,q5 | q6,q7,q1 | q0), so
  attention work unlocks uniformly against the DMA stream and only ~4 beats
  remain after the last byte lands; projection matmuls are interleaved
  between beats (offset past each round's start so not-yet-arrived data
  never stalls the in-order PE queue).
- Output partials are written as bf16 and drained via the idle gpsimd DMA
  queue (a sync-queue drain would block later input DMA issue).
- The boolean mask is block-analyzed on the host: fully-kept 512x128 blocks
  need no mask work, fully-dropped blocks are skipped at compile time,
  partially-kept blocks multiply the exp() output by a 0/1 tile from a tiny
  deduplicated table (2 distinct tiles for a causal mask).
"""

import os
import sys

sys.path.insert(0, "/opt/trn_rl_repo")

import numpy as np
import ml_dtypes

import concourse.bass as bass
import concourse.mybir as mybir
import concourse.tile as tile
from concourse import bacc
from concourse.bass_utils import run_bass_kernel_spmd
from concourse.masks import make_identity

F32 = mybir.dt.float32
BF16 = mybir.dt.bfloat16
BF16_NP = ml_dtypes.bfloat16

N_CORES = 8
B, T, D, H = 4, 4096, 1024, 64
P = 128                      # partitions / k-tile rows
QC = 512                     # query chunk width
NJ = T // QC                 # 8 query chunks
GT = T // P                  # 32 global k-tiles
LT = GT // 2                 # 16 local (per-parity) k-tiles
D_TILES = D // P             # 8
KVW = 512                    # k/v projection chunk width (4 local tiles)
NKV = LT * P // KVW          # 4 kv chunks / emission blocks
# DMA window contents: kv chunks front-loaded, q streamed one chunk per
# window so attention work unlocks uniformly against the DMA stream.
WIN_DMA = [
    [("w",), ("k", 0), ("v", 0), ("q", 2), ("q", 3)],
    [("masks",), ("k", 1), ("v", 1), ("q", 4), ("q", 5)],
    [("k", 2), ("v", 2), ("q", 6), ("q", 7), ("q", 1)],
    [("k", 3), ("v", 3), ("q", 0)],
    [],
]
NWIN = len(WIN_DMA)
KVWIN = [0, 1, 2, 3]             # window whose thunks project kv chunk c
QWIN = [3, 2, 0, 0, 1, 1, 2, 2]  # window whose thunks project q chunk j

LAST_RESULT = None           # test harness reads exec_time_ns from here
_CACHE = {}


def _schedule(mask):
    """Per query chunk j: the list of local k-tile indices both parity cores
    process (compile-time), and per entry the mask-table slot to multiply
    with (None = block fully kept for both parities)."""
    m = mask.reshape(NJ, QC, GT, P)
    blk_any = m.any(axis=(1, 3))   # [j, g]
    blk_all = m.all(axis=(1, 3))
    tidx, mslot, slots = [], [], {}
    for j in range(NJ):
        idxs, ms = [], []
        for i in range(LT):
            g0, g1 = 2 * i, 2 * i + 1
            if not (blk_any[j, g0] or blk_any[j, g1]):
                continue
            idxs.append(i)
            if blk_all[j, g0] and blk_all[j, g1]:
                ms.append(None)
            else:
                key = (mask[j * QC:(j + 1) * QC, g0 * P:(g0 + 1) * P].tobytes(),
                       mask[j * QC:(j + 1) * QC, g1 * P:(g1 + 1) * P].tobytes())
                ms.append(slots.setdefault(key, len(slots)))
        tidx.append(tuple(idxs))
        mslot.append(tuple(ms))
    return tuple(tidx), tuple(mslot), slots


def _mask_tables(mask, tidx, mslot, n_slots):
    """[2][n_slots, 128, 512] bf16 0/1 tiles (per parity)."""
    mp = [np.zeros((max(1, n_slots), P, QC), BF16_NP) for _ in range(2)]
    done = set()
    for j in range(NJ):
        for pos, i in enumerate(tidx[j]):
            s = mslot[j][pos]
            if s is None or s in done:
                continue
            done.add(s)
            for par in range(2):
                g = 2 * i + par
                blk = mask[j * QC:(j + 1) * QC, g * P:(g + 1) * P]
                mp[par][s] = blk.T.astype(BF16_NP)
    return mp


def _beat_blocks(tidx):
    """Assign attention beats (j, ii) to emission windows by data readiness;
    drains follow each chunk's last beat.  Falls back to chunk-sequential
    emission if the readiness-ordered schedule would need >3 concurrent
    PSUM accumulators."""
    ext = [len(t) for t in tidx]
    nbeats = [(e + 1) // 2 for e in ext]

    def win_of(j, ii):
        tiles = tidx[j][ii:ii + 2]
        return max(KVWIN[max(tiles) // (KVW // P)], QWIN[j])

    def entries_sorted():
        beats = []
        for j in range(NJ):
            for ii in range(0, ext[j], 2):
                beats.append((win_of(j, ii), j, ii))
        # within a window: finish already-open chunks first (frees their
        # PSUM accumulator before new chunks open), then alternate new
        # chunks ii-major so their S/exp/PV pipelines interleave
        fw = {}
        for w, j, ii in beats:
            fw[j] = min(fw.get(j, w), w)
        beats.sort(key=lambda t: (t[0], 0 if fw[t[1]] < t[0] else 1,
                                  t[2], t[1]))
        blocks = [[] for _ in range(NWIN)]
        seen = {j: 0 for j in range(NJ)}
        for w, j, ii in beats:
            blocks[w].append(("beat", j, ii))
            seen[j] += 1
            if seen[j] == nbeats[j]:
                blocks[w].append(("drain", j))
        for j in range(NJ):
            if ext[j] == 0:
                blocks[0].append(("zero", j))
                blocks[0].append(("drain", j))
        return blocks

    def ring_ok(blocks, ring=3):
        order = [e for b in blocks for e in b]
        open_order, drains = [], []
        for e in order:
            if e[0] in ("beat", "zero") and e[1] not in open_order:
                open_order.append(e[1])
                if len(open_order) > ring:
                    victim = open_order[len(open_order) - 1 - ring]
                    if victim not in drains:
                        return False
            elif e[0] == "drain":
                drains.append(e[1])
        return True

    blocks = entries_sorted()
    if ring_ok(blocks):
        return blocks
    # fallback: all beats of a chunk in the window where its last tile lands
    blocks = [[] for _ in range(NWIN)]
    for j in range(NJ):
        if ext[j] == 0:
            blocks[0] += [("zero", j), ("drain", j)]
            continue
        w = max(KVWIN[max(tidx[j]) // (KVW // P)], QWIN[j])
        for ii in range(0, ext[j], 2):
            blocks[w].append(("beat", j, ii))
        blocks[w].append(("drain", j))
    return blocks


def _build(tidx, mslot, n_slots):
    n_mask = max(1, n_slots)
    nc = bacc.Bacc("TRN2", target_bir_lowering=False, debug=False,
                   num_devices=N_CORES)
    qT = nc.dram_tensor("qT", [NJ, P, D_TILES, QC], BF16,
                        kind="ExternalInput")
    kT = nc.dram_tensor("kT", [NKV, P, D_TILES, KVW], BF16,
                        kind="ExternalInput")
    vT = nc.dram_tensor("vT", [NKV, P, D_TILES, KVW], BF16,
                        kind="ExternalInput")
    w = nc.dram_tensor("w", [P, D_TILES, 5 * H], BF16, kind="ExternalInput")
    maskp = nc.dram_tensor("maskp", [P, n_mask, QC], BF16,
                           kind="ExternalInput")
    out = nc.dram_tensor("out", [NJ, H + 1, QC], BF16,
                     kind="ExternalOutput")

    Exp = mybir.ActivationFunctionType.Exp
    blocks = _beat_blocks(tidx)

    with tile.TileContext(nc) as tc:
        with (
            tc.tile_pool(name="const", bufs=1) as cpool,
            tc.tile_pool(name="proj", bufs=1) as projpool,
        ):
            w_sb = cpool.tile([P, D_TILES, 5 * H], BF16)
            msk = cpool.tile([P, n_mask, QC], BF16)
            ident = cpool.tile([P, P], F32)

            qhT = projpool.tile([P, T], BF16, tag="qhT")      # qh in both halves
            khT = projpool.tile([P, LT // 2, P], BF16, tag="khT")
            vh = projpool.tile([P, LT, H + 1], BF16, tag="vh")

            with (
                tc.tile_pool(name="xs", bufs=16) as xpool,
                tc.tile_pool(name="pp", bufs=1, space="PSUM") as pppool,
                tc.tile_pool(name="sp", bufs=2, space="PSUM") as spool,
                tc.tile_pool(name="oac", bufs=3, space="PSUM") as opool,
                tc.tile_pool(name="vt", bufs=2) as vtpool,
                tc.tile_pool(name="pt", bufs=4) as ppool,
                tc.tile_pool(name="ost", bufs=2) as ostpool,
            ):
                oaccs = {}

                # ---------- emitter thunks ----------
                def dma_x(src, idx):
                    def go():
                        xt = xpool.tile([P, D_TILES, QC], BF16, tag="x",
                                        name="xt")
                        nc.sync.dma_start(out=xt[:], in_=src.ap()[idx])
                        return xt
                    return go

                def proj_thunks(xt_ref, wlo, whi, m_parts, out_cb, width):
                    """8 matmul thunks accumulating [m_parts, width] then a
                    finisher callback on the psum tile."""
                    state = {}
                    def mk(dt_):
                        def go():
                            if dt_ == 0:
                                state["ps"] = pppool.tile(
                                    [m_parts, width], F32, tag="pp", name="ps")
                            nc.tensor.matmul(
                                state["ps"][:], lhsT=w_sb[:, dt_, wlo:whi],
                                rhs=state["xt"][:, dt_, :],
                                start=(dt_ == 0), stop=(dt_ == D_TILES - 1))
                        return go
                    def first():
                        state["xt"] = xt_ref()
                    thunks = []
                    for dt_ in range(D_TILES):
                        if dt_ == 0:
                            g = mk(0)
                            thunks.append(lambda g=g: (first(), g()))
                        else:
                            thunks.append(mk(dt_))
                    thunks.append(lambda: out_cb(state["ps"]))
                    return thunks

                def q_finish(j):
                    def go(ps):
                        nc.vector.tensor_copy(
                            out=qhT[:, j * QC:(j + 1) * QC], in_=ps[:])
                    return go

                def k_finish(c):
                    def go(ps):
                        for t in range(2):
                            sl = 2 * c + t
                            nc.vector.tensor_copy(
                                out=khT[0:H, sl, :],
                                in_=ps[0:H, 2 * t * P:(2 * t + 1) * P])
                            nc.vector.tensor_copy(
                                out=khT[H:P, sl, :],
                                in_=ps[H:P, (2 * t + 1) * P:(2 * t + 2) * P])
                    return go

                def v_finish(c):
                    def go(ps):
                        vtmp = vtpool.tile([H + 1, KVW], F32, tag="vt",
                                           name="vtmp")
                        nc.vector.tensor_copy(out=vtmp[0:H, :], in_=ps[:])
                        nc.vector.memset(vtmp[H:H + 1, :], 1.0)
                        for tt in range(KVW // P):
                            tp = pppool.tile([P, H + 1], F32, tag="pp",
                                             name="tp")
                            nc.tensor.transpose(
                                tp[:], vtmp[:, tt * P:(tt + 1) * P],
                                ident[0:H + 1, 0:H + 1])
                            nc.vector.tensor_copy(
                                out=vh[:, c * (KVW // P) + tt, :], in_=tp[:])
                    return go

                def emit_beat(j, ii):
                    tiles = tidx[j][ii:ii + 2]
                    pw = len(tiles)
                    ext = len(tidx[j])
                    sp = spool.tile([P, 2 * QC], F32, tag="S", name="sp")
                    for u, i in enumerate(tiles):
                        half = (i % 2) * H
                        nc.tensor.matmul(
                            sp[:, u * QC:(u + 1) * QC],
                            lhsT=khT[half:half + H, i // 2, :],
                            rhs=qhT[half:half + H, j * QC:(j + 1) * QC],
                            start=True, stop=True)
                    pt = ppool.tile([P, 2 * QC], BF16, tag="p", name="pt")
                    nc.scalar.activation(
                        out=pt[:, 0:pw * QC], in_=sp[:, 0:pw * QC],
                        func=Exp, scale=0.125)
                    for u in range(pw):
                        s = mslot[j][ii + u]
                        if s is not None:
                            nc.vector.tensor_mul(
                                pt[:, u * QC:(u + 1) * QC],
                                pt[:, u * QC:(u + 1) * QC],
                                msk[:, s, :])
                    def pv():
                        if ii == 0:
                            oaccs[j] = opool.tile([H + 1, QC], F32,
                                                  tag="oacc", name="oacc")
                        for u, i in enumerate(tiles):
                            nc.tensor.matmul(
                                oaccs[j][:],
                                lhsT=vh[:, i, :],
                                rhs=pt[:, u * QC:(u + 1) * QC],
                                start=(ii + u == 0),
                                stop=(ii + u == ext - 1))
                    return pv

                def emit_drain(j):
                    # drain via the idle gpsimd queue: a sync-queue dma_start
                    # would wait on the result sem and stall later input DMAs
                    # behind it (the sync sequencer is in-order)
                    ost = ostpool.tile([H + 1, QC], BF16, tag="ost",
                                       name="ost")
                    nc.vector.tensor_copy(out=ost[:], in_=oaccs[j][:])
                    nc.gpsimd.dma_start(out=out.ap()[j], in_=ost[:])

                # ---------- emission ----------
                make_identity(nc, ident[:])

                prev_beats = []       # beats of window r-1, emitted in round r
                for r in range(NWIN + 1):
                    # DMAs + projection thunks for this window's data
                    thunks = []
                    for ent in (WIN_DMA[r] if r < NWIN else []):
                        if ent[0] == "w":
                            nc.sync.dma_start(out=w_sb[:], in_=w.ap())
                        elif ent[0] == "masks":
                            nc.sync.dma_start(out=msk[:], in_=maskp.ap())
                        elif ent[0] == "k":
                            c = ent[1]
                            kx = dma_x(kT, c)()
                            thunks += proj_thunks(
                                lambda kx=kx: kx, 2 * H, 4 * H, P,
                                k_finish(c), KVW)
                        elif ent[0] == "v":
                            c = ent[1]
                            vx = dma_x(vT, c)()
                            thunks += proj_thunks(
                                lambda vx=vx: vx, 4 * H, 5 * H, H,
                                v_finish(c), KVW)
                        else:
                            j = ent[1]
                            qx = dma_x(qT, j)()
                            thunks += proj_thunks(
                                lambda qx=qx: qx, 0, 2 * H, P,
                                q_finish(j), QC)
                    # interleave previous window's beats with this one's proj
                    nb = max(1, len([e for e in prev_beats if e[0] == "beat"]))
                    skip = nb // 4   # this round's thunks wait on DMA still
                    ti = 0           # in flight; don't let them stall beats
                    bi = 0
                    for e in prev_beats:
                        if e[0] == "beat":
                            pv = emit_beat(e[1], e[2])
                            bi += 1
                            hi = (len(thunks) * max(0, bi - skip)
                                  // max(1, nb - skip))
                            while ti < hi:
                                thunks[ti]()
                                ti += 1
                            pv()
                        elif e[0] == "zero":
                            oaccs[e[1]] = opool.tile([H + 1, QC], F32,
                                                     tag="oacc", name="oacc")
                            nc.vector.memset(oaccs[e[1]][:], 0.0)
                        else:
                            emit_drain(e[1])
                    while ti < len(thunks):
                        thunks[ti]()
                        ti += 1
                    prev_beats = blocks[r] if r < NWIN else []

    nc.compile()
    return nc


def _get_nc(key, tidx, mslot, n_slots):
    if key not in _CACHE:
        _CACHE[key] = _build(tidx, mslot, n_slots)
    return _CACHE[key]


def _tile_x(x2d, nchunks, width):
    """[D, nchunks*width] -> [nchunks, P, D_TILES, width] contiguous."""
    return np.ascontiguousarray(
        x2d.reshape(D_TILES, P, nchunks, width).transpose(2, 1, 0, 3))


def _make_in_maps(q, k, v, wcat, mp):
    cols = [np.concatenate(
        [np.arange((2 * i + par) * P, (2 * i + par + 1) * P)
         for i in range(LT)]) for par in range(2)]
    in_maps = []
    for c_ in range(N_CORES):
        b, par = divmod(c_, 2)
        qTb = _tile_x(q[b].T.astype(BF16_NP), NJ, QC)
        kTb = _tile_x(k[b].T[:, cols[par]].astype(BF16_NP), NKV, KVW)
        vTb = _tile_x(v[b].T[:, cols[par]].astype(BF16_NP), NKV, KVW)
        in_maps.append({
            "qT": qTb, "kT": kTb, "vT": vTb, "w": wcat,
            "maskp": np.ascontiguousarray(mp[par].transpose(1, 0, 2)),
        })
    return in_maps


def _gather_out(results):
    outp = np.empty((B, T, H), np.float32)
    for b in range(B):
        acc = (results[2 * b]["out"].astype(np.float32)
               + results[2 * b + 1]["out"].astype(np.float32))
        num = acc[:, 0:H, :]
        den = acc[:, H, :]
        outp[b] = (np.moveaxis(num, 1, 2) / den[:, :, None]).reshape(T, H)
    return outp


def kernel(q, k, v, Wq, Wk, Wv, attn_mask):
    global LAST_RESULT
    q = np.asarray(q, dtype=np.float32)
    k = np.asarray(k, dtype=np.float32)
    v = np.asarray(v, dtype=np.float32)
    mask = np.asarray(attn_mask).astype(bool)
    Wq = np.asarray(Wq, np.float32)
    Wk = np.asarray(Wk, np.float32)
    Wv = np.asarray(Wv, np.float32)
    # [Wq|Wq|Wk|Wk|Wv]: duplicated halves put qh/kh in both partition halves
    wcat = np.concatenate([Wq, Wq, Wk, Wk, Wv], axis=1).astype(BF16_NP)
    wcat = np.ascontiguousarray(
        wcat.reshape(D_TILES, P, 5 * H).transpose(1, 0, 2))

    tidx, mslot, slots = _schedule(mask)
    key = (tidx, mslot, len(slots))
    nc = _get_nc(key, tidx, mslot, len(slots))
    mp = _mask_tables(mask, tidx, mslot, len(slots))
    in_maps = _make_in_maps(q, k, v, wcat, mp)

    res = run_bass_kernel_spmd(
        nc, in_maps, core_ids=list(range(N_CORES)),
        trace=bool(os.environ.get("KBENCH_TRACE")))
    LAST_RESULT = res
    return _gather_out(res.results)
